# revision 1
# baseline (speedup 1.0000x reference)
"""AttentionBlock (GroupNorm -> QKV -> 8-head attention -> proj -> residual)
as a Bass/Tile kernel for Trainium2, data-parallel over batch on 8 cores.

Self-contained: hardcodes shapes B=8, C=512, H=W=32 (N=1024), heads=8, d=64,
groups=32.  Each core processes one batch element; all params replicated.

Layout strategy (per core):
  x [C, N] channel-major -> 4 SBUF tiles [128, 1024].
  GroupNorm: per-channel mean/var via bn_stats/bn_aggr, then cross-partition
  group aggregation + broadcast via two tiny mask matmuls on the PE.
  QKV: out = wqkvT.T @ xn accumulated over 4 K-tiles -> 12 tiles [128, 1024]
  (Q tiles 0-3, K 4-7, V 8-11; head h at partition offset (h%2)*64 of tile h//2).
  Attention (per head, transposed orientation; no max-subtraction, exp is safe):
    scoresT[k, q] = K^T Q via PE (K on partitions), exp via ACT (scale=1/8),
    contextT accumulated as vT^T @ probsT where vT carries a 64-wide ones
    block, so rows 64-127 of the accumulator hold the softmax denominator per
    query (broadcast for free).  Normalize with DVE reciprocal+multiply ->
    h_attT tiles [128, 1024] (channels on partitions).
  proj: wprojT.T @ h_attT + proj_b + x -> out.

Matmul inputs are float32r (same bits as fp32, reduced-precision PE mode):
fp32 matmuls stream at 4 cycles/row, f32r at 1 cycle/row for N>=256.  The BIR
verifier requires f32r matmul operands to be *produced* as f32r, so every
tile feeding a matmul is allocated as f32r.

Scheduling: serial phases (GN -> all QKV -> v transposes -> attention pairs
-> proj) keep the PE stream dense.  Within an attention pair the context
matmuls run LAG=2 k-tiles behind the scores/exp so the PE never waits on an
ACT exp result.  1/denominator is computed on the ACT engine as exp(-ln(d)).
PSUM is budgeted exactly 8 banks: score tiles 2x2 (slots shared with the
QKV/proj half-N accumulators and v-transposes, which run in other phases),
context accumulators 2x2.
"""

import sys

sys.path.insert(0, "/opt/trn_rl_repo")

import numpy as np

B, C, HH, WW = 8, 512, 32, 32
N = HH * WW          # 1024
NH, HD = 8, 64       # heads, head dim
NG = 32              # groupnorm groups
EPS = 1e-5
NT = C // 128        # 4 channel tiles
MT = 16              # qkv m-tiles: Q 0-3 | K-padded 4-11 | V 12-15
KT = N // 128        # 8 key tiles
NCORES = 8

_CACHE: dict = {}


def _build_program():
    import concourse.bacc as bacc
    import concourse.tile as tile
    from concourse import mybir

    f32 = mybir.dt.float32
    f32r = mybir.dt.float32r
    bf16 = mybir.dt.bfloat16
    AF = mybir.ActivationFunctionType
    OP = mybir.AluOpType

    nc = bacc.Bacc("TRN2", target_bir_lowering=False, debug=False)

    x_d = nc.dram_tensor("x", [C, N], f32, kind="ExternalInput").ap()
    wqkv_d = nc.dram_tensor("wqkvT", [C, MT * 128], f32r, kind="ExternalInput").ap()
    wproj_d = nc.dram_tensor("wprojT", [C, C], f32r, kind="ExternalInput").ap()
    bqkv_d = nc.dram_tensor("bqkv", [128, MT], f32, kind="ExternalInput").ap()
    bproj_d = nc.dram_tensor("bproj", [128, NT], f32, kind="ExternalInput").ap()
    gnw_d = nc.dram_tensor("gnw", [128, NT], f32, kind="ExternalInput").ap()
    gnb_d = nc.dram_tensor("gnb", [128, NT], f32, kind="ExternalInput").ap()
    gmask_d = nc.dram_tensor("gmask", [128, 8], f32, kind="ExternalInput").ap()
    gmaskT_d = nc.dram_tensor("gmaskT", [8, 128], f32, kind="ExternalInput").ap()
    ident_d = nc.dram_tensor("ident2", [128, HD], f32r, kind="ExternalInput").ap()
    ones_d = nc.dram_tensor("ones64", [128, HD], f32r, kind="ExternalInput").ap()
    out_d = nc.dram_tensor("out", [C, N], f32, kind="ExternalOutput").ap()

    x_dt = x_d.rearrange("(t p) n -> t p n", p=128)
    out_dt = out_d.rearrange("(t p) n -> t p n", p=128)
    wq_dt = wqkv_d.rearrange("(t p) m -> t p m", p=128)
    wp_dt = wproj_d.rearrange("(t p) m -> t p m", p=128)

    from contextlib import ExitStack

    with tile.TileContext(nc) as tc, ExitStack() as ctx:
        sg = ctx.enter_context(tc.tile_pool(name="sg", bufs=1))
        work = ctx.enter_context(tc.tile_pool(name="work", bufs=1))
        pb_pool = ctx.enter_context(tc.tile_pool(name="pbp", bufs=6))
        small = ctx.enter_context(tc.tile_pool(name="small", bufs=4))
        outp = ctx.enter_context(tc.tile_pool(name="outp", bufs=2))
        # PSUM budget (8 banks): "sc" slots 2x2 banks (scores; also reused by
        # the QKV/proj half-accumulators, v-transposes and GN matmuls, which
        # run in phases where scores are idle), "cx" slots 2x2 banks (the two
        # live context accumulators of a head pair).
        psc = ctx.enter_context(tc.tile_pool(name="psc", bufs=2, space="PSUM"))
        pcx = ctx.enter_context(tc.tile_pool(name="pcx", bufs=2, space="PSUM"))

        # ---- load x, small constants, then weights ----
        x_sb = []
        for t in range(NT):
            xt = work.tile([128, N], f32, name=f"x{t}", tag=f"x{t}")
            nc.sync.dma_start(out=xt[:, 0:512], in_=x_dt[t][:, 0:512])
            nc.sync.dma_start(out=xt[:, 512:1024], in_=x_dt[t][:, 512:1024])
            x_sb.append(xt)

        bqkv_sb = sg.tile([128, MT], f32, name="bqkv_sb")
        nc.sync.dma_start(out=bqkv_sb, in_=bqkv_d)
        bproj_sb = sg.tile([128, NT], f32, name="bproj_sb")
        nc.sync.dma_start(out=bproj_sb, in_=bproj_d)
        gnw_sb = sg.tile([128, NT], f32, name="gnw_sb")
        nc.sync.dma_start(out=gnw_sb, in_=gnw_d)
        gnb_sb = sg.tile([128, NT], f32, name="gnb_sb")
        nc.sync.dma_start(out=gnb_sb, in_=gnb_d)
        gmask_sb = sg.tile([128, 8], f32, name="gmask_sb")
        nc.sync.dma_start(out=gmask_sb, in_=gmask_d)
        gmaskT_sb = sg.tile([8, 128], f32, name="gmaskT_sb")
        nc.sync.dma_start(out=gmaskT_sb, in_=gmaskT_d)
        ident_sb = sg.tile([128, HD], f32r, name="ident_sb")
        nc.sync.dma_start(out=ident_sb, in_=ident_d)
        ones_sb = sg.tile([128, HD], f32r, name="ones_sb")
        nc.sync.dma_start(out=ones_sb, in_=ones_d)

        wq_sb = []
        for t in range(NT):
            wt = sg.tile([128, MT * 128], f32r, name=f"wq{t}", tag=f"wq{t}")
            nc.sync.dma_start(out=wt, in_=wq_dt[t])
            wq_sb.append(wt)
        wp_sb = []
        for t in range(NT):
            wt = sg.tile([128, C], f32r, name=f"wp{t}", tag=f"wp{t}")
            nc.sync.dma_start(out=wt, in_=wp_dt[t])
            wp_sb.append(wt)

        ident_bf = sg.tile([128, HD], bf16, name="ident_bf")
        nc.vector.tensor_copy(ident_bf, ident_sb)
        ones_bf = sg.tile([128, HD], bf16, name="ones_bf")
        nc.vector.tensor_copy(ones_bf, ones_sb)

        eps_sb = sg.tile([8, 1], f32, name="eps_sb")
        nc.vector.memset(eps_sb, EPS)

        # ---- GroupNorm statistics ----
        # per-channel mean/var over N, then group-aggregate across partitions.
        allstats = sg.tile([128, 2 * NT], f32, name="allstats")
        for t in range(NT):
            bns = small.tile([128, 2, 6], f32, name=f"bns{t}", tag="bns")
            nc.vector.bn_stats(out=bns[:, 0, :], in_=x_sb[t][:, 0:512])
            nc.vector.bn_stats(out=bns[:, 1, :], in_=x_sb[t][:, 512:1024])
            nc.vector.bn_aggr(out=allstats[:, 2 * t : 2 * t + 2], in_=bns)
            # E[x^2] = var + mean^2 into the odd column
            m2 = small.tile([128, 1], f32, name=f"m2_{t}", tag="m2")
            nc.vector.tensor_mul(
                m2, allstats[:, 2 * t : 2 * t + 1], allstats[:, 2 * t : 2 * t + 1]
            )
            nc.vector.tensor_add(
                allstats[:, 2 * t + 1 : 2 * t + 2],
                allstats[:, 2 * t + 1 : 2 * t + 2],
                m2,
            )

        # group aggregate: [8 local groups, 2*NT stats]
        grp_ps = psc.tile([8, 2 * NT], f32, name="grp_ps", tag="sc")
        nc.tensor.matmul(grp_ps, gmask_sb, allstats)
        grp_sb = sg.tile([8, 2 * NT], f32, name="grp_sb")
        nc.vector.tensor_copy(grp_sb, grp_ps)
        # var = E[x^2] - mean^2 ; rstd = 1/sqrt(var+eps)  (in cols 1::2)
        msq = sg.tile([8, NT], f32, name="msq")
        nc.vector.tensor_mul(msq, grp_sb[:, 0 : 2 * NT : 2], grp_sb[:, 0 : 2 * NT : 2])
        nc.vector.tensor_sub(
            grp_sb[:, 1 : 2 * NT : 2], grp_sb[:, 1 : 2 * NT : 2], msq
        )
        nc.scalar.activation(
            out=grp_sb[:, 1 : 2 * NT : 2],
            in_=grp_sb[:, 1 : 2 * NT : 2],
            func=AF.Sqrt,
            bias=eps_sb,
            scale=1.0,
        )
        nc.vector.reciprocal(grp_sb[:, 1 : 2 * NT : 2], grp_sb[:, 1 : 2 * NT : 2])

        # broadcast group stats back to channels: [128, 2*NT]
        chan_ps = psc.tile([128, 2 * NT], f32, name="chan_ps", tag="sc")
        nc.tensor.matmul(chan_ps, gmaskT_sb, grp_sb)
        chan_sb = sg.tile([128, 2 * NT], f32, name="chan_sb")
        nc.vector.tensor_copy(chan_sb, chan_ps)

        # A = rstd * gn_w ; Bc = gn_b - mean * A   (per channel, per tile col)
        A_sb = sg.tile([128, NT], f32, name="A_sb")
        nc.vector.tensor_mul(A_sb, chan_sb[:, 1 : 2 * NT : 2], gnw_sb)
        B_sb = sg.tile([128, NT], f32, name="B_sb")
        nc.vector.tensor_mul(B_sb, chan_sb[:, 0 : 2 * NT : 2], A_sb)
        nc.vector.tensor_sub(B_sb, gnb_sb, B_sb)

        # ---- GN apply ----
        xn_sb = []
        for t in range(NT):
            xn = work.tile([128, N], f32r, name=f"xn{t}", tag=f"xn{t}")
            nc.vector.tensor_scalar(
                out=xn,
                in0=x_sb[t],
                scalar1=A_sb[:, t : t + 1],
                scalar2=B_sb[:, t : t + 1],
                op0=OP.mult,
                op1=OP.add,
            )
            xn_sb.append(xn)

        # ---- QKV m-tile builder (half-N PSUM accumulators: 1 bank each) ----
        qkv_sb = [None] * MT

        def qkv_mtile(mt):
            qt = work.tile([128, N], bf16, name=f"qkv{mt}", tag=f"qkv{mt}")
            for hlf in range(2):
                qpool, qtag = (psc, "sc") if hlf == 0 else (pcx, "cx")
                qp = qpool.tile([128, 512], f32, name=f"qp{mt}_{hlf}", tag=qtag)
                for kc in range(NT):
                    nc.tensor.matmul(
                        qp,
                        wq_sb[kc][:, mt * 128 : (mt + 1) * 128],
                        xn_sb[kc][:, hlf * 512 : (hlf + 1) * 512],
                        start=(kc == 0),
                        stop=(kc == NT - 1),
                    )
                nc.vector.tensor_scalar_add(
                    qt[:, hlf * 512 : (hlf + 1) * 512], qp, bqkv_sb[:, mt : mt + 1]
                )
            qkv_sb[mt] = qt

        # vT tiles [128 keys, 128]: cols 0-63 = v^T, cols 64-127 = ones.  The
        # ones block makes the context matmul broadcast the per-query softmax
        # denominator into output rows 64-127 for free (same streaming cycles).
        vt_sb = [[None] * KT for _ in range(NH)]

        def v_transposes_pair(j):
            # PE transposes (bf16, 1 cycle/row); adjacent transposes for the
            # head pair land in disjoint PE row groups.
            for kt in range(KT):
                tps = {}
                for h in (2 * j, 2 * j + 1):
                    po = (h % 2) * HD
                    v_ap = qkv_sb[12 + h // 2][po : po + HD, :]
                    idt = ident_bf[po : po + HD, :]
                    tpool, ttag = (psc, "sc") if h % 2 == 0 else (pcx, "cx")
                    tp = tpool.tile([128, HD], bf16, name=f"tp{h}_{kt}", tag=ttag)
                    nc.tensor.transpose(tp, v_ap[:, kt * 128 : (kt + 1) * 128], idt)
                    tps[h] = tp
                for h in (2 * j, 2 * j + 1):
                    vt = work.tile([128, 128], bf16, name=f"vt{h}_{kt}", tag=f"vt{h}_{kt}")
                    nc.vector.tensor_copy(vt[:, 0:HD], tps[h])
                    nc.vector.tensor_copy(vt[:, HD:128], ones_bf)
                    vt_sb[h][kt] = vt

        hatt_sb = []
        for t in range(NT):
            ht = work.tile([128, N], f32r, name=f"hatt{t}", tag=f"hatt{t}")
            hatt_sb.append(ht)

        def attn_pair(j):
            h0, h1 = 2 * j, 2 * j + 1
            ap = {}
            cx = {}
            for h in (h0, h1):
                # q: full 128-partition tile (head pair); k: per-head tile
                # whose other 64 partition rows are ZERO (host weight layout),
                # so the K=128 contraction ignores the other head's q rows.
                # K=128 matters: K=64 matmuls stream at half rate on TRN2.
                ap[h] = (qkv_sb[h // 2], qkv_sb[4 + h])
                cx[h] = pcx.tile([128, N], f32, name=f"cx{h}", tag="cx")

            # software pipeline: scores/exp run LAG k-tiles ahead of the
            # context accumulation, so the PE never waits on an exp result
            # and stays dense (keeps the HAM clock-gate warm).
            LAG = 2
            pbs = {}

            def emit_sc(kt):
                sc = {}
                for h in (h0, h1):
                    sc[h] = psc.tile([128, N], f32, name=f"sc{h}_{kt}", tag="sc")
                for h in (h0, h1):
                    q_ap, k_ap = ap[h]
                    lhsT = k_ap[:, kt * 128 : (kt + 1) * 128]
                    for hlf in range(2):
                        nc.tensor.matmul(
                            sc[h][:, hlf * 512 : (hlf + 1) * 512],
                            lhsT,
                            q_ap[:, hlf * 512 : (hlf + 1) * 512],
                        )
                for h in (h0, h1):
                    pb = pb_pool.tile([128, N], bf16, name=f"pb{h}_{kt}", tag="pb")
                    nc.scalar.activation(
                        out=pb, in_=sc[h], func=AF.Exp, scale=1.0 / 8.0
                    )
                    pbs[(h, kt)] = pb

            def emit_cx(kt):
                for h in (h0, h1):
                    for hlf in range(2):
                        nc.tensor.matmul(
                            cx[h][:, hlf * 512 : (hlf + 1) * 512],
                            vt_sb[h][kt],
                            pbs[(h, kt)][:, hlf * 512 : (hlf + 1) * 512],
                            start=(kt == 0),
                            stop=(kt == KT - 1),
                        )

            for kt in range(KT + LAG):
                if kt < KT:
                    emit_sc(kt)
                if kt >= LAG:
                    emit_cx(kt - LAG)

            # rows 64-127 of cx hold the softmax denominator per query.
            # 1/d = exp(-ln(d)) on the ACT engine; both heads' Ln ops emitted
            # back-to-back so the Exp<->Ln LUT set switches only twice per
            # pair (a switch costs ~1.3us).
            lnd = {}
            rsb = {}
            for h in (h0, h1):
                lnd[h] = small.tile([HD, N], f32, name=f"lnd{h}", tag="lnd", bufs=2)
                nc.scalar.activation(out=lnd[h], in_=cx[h][HD:128, :], func=AF.Ln)
            for h in (h0, h1):
                rsb[h] = small.tile([HD, N], f32, name=f"rsb{h}", tag="rsb", bufs=2)
                nc.scalar.activation(out=rsb[h], in_=lnd[h], func=AF.Exp, scale=-1.0)
            for h in (h0, h1):
                po = (h % 2) * HD
                nc.vector.tensor_mul(
                    hatt_sb[h // 2][po : po + HD, :], cx[h][0:HD, :], rsb[h]
                )

        # ---- serial phases: QKV + transposes dense on the PE, then the
        # attention pairs; each pair's normalize is deferred into the next
        # pair's stream so the ACT exp feed is never blocked ----
        for mt in range(MT):
            qkv_mtile(mt)
        for j in range(NH // 2):
            v_transposes_pair(j)
        for j in range(NH // 2):
            attn_pair(j)

        # ---- proj + bias + residual (half-N accumulators) ----
        for mt in range(NT):
            ot = outp.tile([128, N], f32, name=f"ot{mt}", tag="ot")
            for hlf in range(2):
                ppool, ptag = (psc, "sc") if hlf == 0 else (pcx, "cx")
                pp = ppool.tile([128, 512], f32, name=f"pp{mt}_{hlf}", tag=ptag)
                for kc in range(NT):
                    nc.tensor.matmul(
                        pp,
                        wp_sb[kc][:, mt * 128 : (mt + 1) * 128],
                        hatt_sb[kc][:, hlf * 512 : (hlf + 1) * 512],
                        start=(kc == 0),
                        stop=(kc == NT - 1),
                    )
                nc.vector.scalar_tensor_tensor(
                    out=ot[:, hlf * 512 : (hlf + 1) * 512],
                    in0=pp,
                    scalar=bproj_sb[:, mt : mt + 1],
                    in1=x_sb[mt][:, hlf * 512 : (hlf + 1) * 512],
                    op0=OP.add,
                    op1=OP.add,
                )
            nc.sync.dma_start(out=out_dt[mt], in_=ot)

    nc.compile()
    return nc


def _get_nc():
    if "nc" not in _CACHE:
        _CACHE["nc"] = _build_program()
    return _CACHE["nc"]


def _host_inputs(x, gn_w, gn_b, qkv_w, qkv_b, proj_w, proj_b):
    f32 = np.float32
    x = np.asarray(x, dtype=f32).reshape(B, C, N)
    gn_w = np.asarray(gn_w, dtype=f32)
    gn_b = np.asarray(gn_b, dtype=f32)
    qkv_w = np.asarray(qkv_w, dtype=f32)
    qkv_b = np.asarray(qkv_b, dtype=f32)
    proj_w = np.asarray(proj_w, dtype=f32)
    proj_b = np.asarray(proj_b, dtype=f32)

    # device QKV weight layout: [W_q (512 cols) | K-padded (1024 cols: per
    # head h a 128-col block, k_h placed at rows (h%2)*64 and zeros in the
    # other 64 rows) | W_v (512 cols)]
    wq_T = qkv_w[0:512].T                      # [C, 512]
    wk_T = qkv_w[512:1024].T                   # [C, 512]
    wv_T = qkv_w[1024:1536].T                  # [C, 512]
    kpad = np.zeros((512, 1024), np.float32)
    bk = qkv_b[512:1024]
    bkpad = np.zeros(1024, np.float32)
    for h in range(8):
        po = (h % 2) * 64
        kpad[:, h * 128 + po : h * 128 + po + 64] = wk_T[:, h * 64 : (h + 1) * 64]
        bkpad[h * 128 + po : h * 128 + po + 64] = bk[h * 64 : (h + 1) * 64]
    wqkvT = np.ascontiguousarray(np.concatenate([wq_T, kpad, wv_T], axis=1))
    bqkv_flat = np.concatenate([qkv_b[0:512], bkpad, qkv_b[1024:1536]])
    wprojT = np.ascontiguousarray(proj_w.T)
    bqkv = np.ascontiguousarray(bqkv_flat.reshape(MT, 128).T)
    bproj = np.ascontiguousarray(proj_b.reshape(NT, 128).T)
    gnw = np.ascontiguousarray(gn_w.reshape(NT, 128).T)
    gnb = np.ascontiguousarray(gn_b.reshape(NT, 128).T)

    p = np.arange(128)
    gmask = np.zeros((128, 8), f32)
    gmask[p, p // 16] = 1.0 / 16.0
    gmaskT = np.ascontiguousarray(
        (np.arange(128)[:, None] // 16 == np.arange(8)[None, :]).astype(f32).T
    )
    ident2 = np.ascontiguousarray(np.tile(np.eye(HD, dtype=f32), (2, 1)))

    common = dict(
        wqkvT=wqkvT, wprojT=wprojT, bqkv=bqkv, bproj=bproj,
        gnw=gnw, gnb=gnb, gmask=gmask, gmaskT=gmaskT, ident2=ident2,
        ones64=np.ones((128, HD), f32),
    )
    return [dict(common, x=np.ascontiguousarray(x[b])) for b in range(B)]


def _run(in_maps, trace=False, **kw):
    from concourse.bass_utils import run_bass_kernel_spmd

    nc = _get_nc()
    return run_bass_kernel_spmd(nc, in_maps, list(range(NCORES)), trace=trace, **kw)


def kernel(x, gn_w, gn_b, qkv_w, qkv_b, proj_w, proj_b):
    in_maps = _host_inputs(x, gn_w, gn_b, qkv_w, qkv_b, proj_w, proj_b)
    res = _run(in_maps)
    out = np.stack([res.results[b]["out"] for b in range(B)])
    return out.reshape(B, C, HH, WW).astype(np.float32)

